# revision 1
# baseline (speedup 1.0000x reference)
"""Per-donor routed linear layer on 8 Trainium2 cores.

out[i] = x[i] @ W[donor_labels[i]].T + b[donor_labels[i]]

Strategy: route on host (stable sort of rows by donor label), one donor per
core, zero-padded to a common row count R.  Each core runs a dense
(R x 1024) @ (1024 x 100) matmul with K tiled 8x128, accumulating in PSUM,
bias added on the DVE during PSUM->SBUF eviction.  x is uploaded
feature-major (transposed on host) so every DMA is contiguous >= 2KB runs;
the output comes back gene-major and is transposed/unpermuted on host.
"""

import os
import sys

sys.path.insert(0, "/opt/trn_rl_repo")

import numpy as np

import concourse.bacc as bacc
import concourse.mybir as mybir
from concourse.tile import TileContext
from concourse.bass_utils import run_bass_kernel_spmd

N_CORES = 8
N_DONORS = 8
D_IN = 1024
N_GENES = 100
K_TILES = D_IN // 128
BLOCK = 512  # moving rows per matmul (one fp32 PSUM bank)

# "f32" = exact fp32 matmul (4 cyc/row); "f32r" = fast fp32 (1 cyc/row)
MM_DTYPE = os.environ.get("BEC_MM_DTYPE", "f32r")


def _build_program(R: int):
    nc = bacc.Bacc(
        "TRN2",
        target_bir_lowering=False,
        debug=False,
        enable_asserts=False,
        num_devices=N_CORES,
    )
    xt = nc.dram_tensor("xt", (D_IN, R), mybir.dt.float32, kind="ExternalInput").ap()
    wt = nc.dram_tensor(
        "wt", (D_IN, N_GENES), mybir.dt.float32, kind="ExternalInput"
    ).ap()
    bias = nc.dram_tensor(
        "bias", (N_GENES, 1), mybir.dt.float32, kind="ExternalInput"
    ).ap()
    outt = nc.dram_tensor(
        "outt", (N_GENES, R), mybir.dt.float32, kind="ExternalOutput"
    ).ap()

    mm_dt = mybir.dt.float32r if MM_DTYPE == "f32r" else mybir.dt.float32

    xt_v = xt.rearrange("(k p) r -> p k r", p=128)  # (128, K_TILES, R)
    wt_v = wt.rearrange("(k p) n -> p k n", p=128)  # (128, K_TILES, N_GENES)

    n_blocks = R // BLOCK

    with TileContext(nc) as tc:
        with (
            tc.tile_pool(name="const", bufs=1) as const_pool,
            tc.tile_pool(name="xp", bufs=4) as x_pool,
            tc.tile_pool(name="op", bufs=4) as out_pool,
            tc.tile_pool(name="ps", bufs=4, space="PSUM") as psum_pool,
        ):
            w_tile = const_pool.tile([128, K_TILES, N_GENES], mybir.dt.float32)
            nc.sync.dma_start(out=w_tile[:], in_=wt_v[:])
            b_tile = const_pool.tile([N_GENES, 1], mybir.dt.float32)
            nc.sync.dma_start(out=b_tile[:], in_=bias[:])

            for j in range(n_blocks):
                x_tile = x_pool.tile([128, K_TILES, BLOCK], mybir.dt.float32)
                nc.sync.dma_start(
                    out=x_tile[:], in_=xt_v[:, :, j * BLOCK : (j + 1) * BLOCK]
                )
                psum = psum_pool.tile([N_GENES, BLOCK], mybir.dt.float32)
                for k in range(K_TILES):
                    nc.tensor.matmul(
                        out=psum[:],
                        lhsT=w_tile[:, k, :].bitcast(mm_dt),
                        rhs=x_tile[:, k, :].bitcast(mm_dt),
                        start=(k == 0),
                        stop=(k == K_TILES - 1),
                    )
                o_tile = out_pool.tile([N_GENES, BLOCK], mybir.dt.float32)
                nc.vector.tensor_scalar_add(out=o_tile[:], in0=psum[:], scalar1=b_tile[:])
                nc.sync.dma_start(
                    out=outt[:, j * BLOCK : (j + 1) * BLOCK], in_=o_tile[:]
                )

    nc.compile()
    return nc


def kernel(x, donor_labels, W, b):
    x = np.ascontiguousarray(x, dtype=np.float32)
    labels = np.asarray(donor_labels).astype(np.int64)
    W = np.asarray(W, dtype=np.float32)
    b = np.asarray(b, dtype=np.float32)
    B = x.shape[0]

    order = np.argsort(labels, kind="stable")
    counts = np.bincount(labels, minlength=N_DONORS)
    starts = np.zeros(N_DONORS + 1, dtype=np.int64)
    np.cumsum(counts, out=starts[1:])
    R = max(BLOCK, int(-(-counts.max() // BLOCK)) * BLOCK)

    in_maps = []
    idx_per_core = []
    for d in range(N_CORES):
        idx = order[starts[d] : starts[d + 1]]
        idx_per_core.append(idx)
        xt = np.zeros((D_IN, R), dtype=np.float32)
        xt[:, : len(idx)] = x[idx].T
        in_maps.append(
            {
                "xt": xt,
                "wt": np.ascontiguousarray(W[d].T),
                "bias": np.ascontiguousarray(b[d].reshape(N_GENES, 1)),
            }
        )

    nc = _build_program(R)
    res = run_bass_kernel_spmd(nc, in_maps, core_ids=list(range(N_CORES)))

    out = np.empty((B, N_GENES), dtype=np.float32)
    for d in range(N_CORES):
        idx = idx_per_core[d]
        out[idx] = res.results[d]["outt"][:, : len(idx)].T
    return out
